# revision 4
# baseline (speedup 1.0000x reference)
"""FPS kernel builder v1 — fused custom-DVE passes.

Per batch per iteration (4 DVE passes over [128, FREE]):
  P1  u    = (x-cx)^2 + (y-cy)^2          SQDIFF2_FPS
  P2  d    = (z-cz)^2 + u                 SQADD_FPS   (matches JAX op order)
  P3  dist = min(dist,d); pmax = rowmax   MINMAX_FPS  (accum)
  P4  score= eq(dist,pmax)*(N-j); pjs=max EQSCORE_FPS (accum)
Cross-partition (packed over batches):
  M = allred_max(pmax); mpj = eq(pmax,M)*pjs; S = allred_max(mpj)
  j* = N - S; mrow = eq(mpj, S)
Gather (PE): wr = [C chunks | score chunks]^T @ mrow; oht = eq(wr_score, S)
  cen = sum_h oht_h^T @ wr_coords_h; bc = ones (x) cen
Emit (GPSIMD): out2d += eq(iota2d, t_col) * j*
"""

import numpy as np
from contextlib import ExitStack

from concourse import bass, tile, mybir, bass_isa

f32 = mybir.dt.float32
i32 = mybir.dt.int32
Alu = mybir.AluOpType
Act = mybir.ActivationFunctionType
RO = bass_isa.ReduceOp
AX = mybir.AxisListType

_OPS = {}


def register_fps_ops():
    if _OPS:
        return _OPS
    from concourse import dve_ops
    from concourse.dve_spec import Spec, Src0, Src1, C0, C1, sq, minn, maxx, eq, lower
    from concourse.dve_uop import DveOpSpec

    def make_op(name, spec):
        for op in dve_ops.OPS:
            if op.name == name:
                return op
        op = dve_ops.DveOp.__new__(dve_ops.DveOp)
        object.__setattr__(op, "name", name)
        object.__setattr__(op, "spec", spec)
        object.__setattr__(op, "subdim", False)
        object.__setattr__(op, "uops_sha", {})
        object.__setattr__(op, "perf_en", {})
        dve_ops.OPS.append(op)
        dve_ops.CUSTOM_DVE_SPECS[name] = spec
        dve_ops._SUB_OPCODE_FOR_NAME[name] = (
            dve_ops._CUSTOM_DVE_ROW_BASE + len(dve_ops.OPS) - 1
        )
        for ver in ("v3", "v4"):
            s = DveOpSpec(name=name, opcode=dve_ops.get_dve_sub_opcode(name),
                          uops=lower(spec, ver=ver),
                          rd1_en=dve_ops.has_src1(spec))
            op.uops_sha[ver] = s.sha(ver)
        return op

    def _accref(fn):
        def r(in0, in1, s0, s1, imm2):
            b = fn(in0, in1, s0, s1, imm2)
            return b, b.reshape(b.shape[0], -1).max(axis=-1, keepdims=True)
        return r

    _OPS["SQDIFF2"] = make_op("SQDIFF2_FPS", Spec(
        body=sq(Src0 - C0) + sq(Src1 - C1),
        reference=lambda in0, in1, s0, s1, imm2:
            (in0 - s0) * (in0 - s0) + (in1 - s1) * (in1 - s1),
    ))
    _OPS["SQADD"] = make_op("SQADD_FPS", Spec(
        body=sq(Src0 - C0) + Src1,
        reference=lambda in0, in1, s0, s1, imm2:
            (in0 - s0) * (in0 - s0) + in1,
    ))
    _OPS["MINMAX"] = make_op("MINMAX_FPS", Spec(
        body=minn(Src0, Src1), accum=maxx,
        reference=_accref(lambda in0, in1, s0, s1, imm2: np.minimum(in0, in1)),
    ))
    _OPS["EQSCORE"] = make_op("EQSCORE_FPS", Spec(
        body=eq(Src0, C0) * Src1, accum=maxx,
        reference=_accref(lambda in0, in1, s0, s1, imm2:
                          (in0 == s0).astype(np.float32) * in1),
    ))
    return _OPS


def fps_ref_np(cloud: np.ndarray, npts: int) -> np.ndarray:
    B, N, _ = cloud.shape
    idx = np.zeros((B, npts), np.int64)
    for b in range(B):
        dist = np.full(N, 1e10, np.float32)
        far = 0
        for t in range(npts):
            idx[b, t] = far
            c = cloud[b, far]
            dx = cloud[b, :, 0] - c[0]
            dy = cloud[b, :, 1] - c[1]
            dz = cloud[b, :, 2] - c[2]
            d = (dx * dx + dy * dy) + dz * dz
            dist = np.minimum(dist, d)
            far = int(np.argmax(dist))
    return idx


def decode_rf(rf: np.ndarray, N: int) -> np.ndarray:
    """rf [nb, 2, NPTS] int32 (R, F planes) -> indices [nb, NPTS] int64."""
    FREE = N // 128
    R = rf[:, 0, :].astype(np.int64)
    F = rf[:, 1, :].astype(np.int64)
    return (128 - R) * FREE + (FREE - F)


def build_fps(tc, out_idx_d, pred, nb: int, N: int, NPTS: int):
    ops = register_fps_ops()
    nc = tc.nc
    FREE = N // 128
    SLOTS = (NPTS + 127) // 128
    PMAX = NPTS // SLOTS
    W = min(FREE, 128)
    NCH = FREE // W
    assert 128 * FREE == N and NCH * W == FREE and PMAX * SLOTS == NPTS

    with ExitStack() as ctx:
        pool = ctx.enter_context(tc.tile_pool(name="main", bufs=1))
        psum = ctx.enter_context(tc.tile_pool(name="ps", bufs=2, space="PSUM"))

        C = [pool.tile([128, 3 * FREE], f32, name=f"C{b}") for b in range(nb)]
        dist = [pool.tile([128, FREE], f32, name=f"dist{b}") for b in range(nb)]
        U = [pool.tile([128, FREE], f32, name=f"U{b}") for b in range(nb)]
        Dd = [pool.tile([128, FREE], f32, name=f"Dd{b}") for b in range(nb)]
        out2d = [pool.tile([128, SLOTS], f32, name=f"out2d{b}") for b in range(nb)]
        out2dF = [pool.tile([128, SLOTS], f32, name=f"out2dF{b}") for b in range(nb)]
        outi = [pool.tile([128, 2 * SLOTS], i32, name=f"outi{b}") for b in range(nb)]
        NC4 = 4 * NCH
        wr = [pool.tile([128, NC4], f32, name=f"wr{b}") for b in range(nb)]
        cen_sb = [pool.tile([1, 3], f32, name=f"cen_sb{b}") for b in range(nb)]
        bcp = [psum.tile([128, 3], f32, tag=f"bcp{b}", name=f"bcp{b}", bufs=1)
               for b in range(nb)]

        NG = 4                      # stagger groups (per-batch chains)
        MAXU = 16                   # loop unroll
        GB = nb // NG               # batches per group
        assert nb % NG == 0 and GB == 1
        pmax4 = [pool.tile([128, GB], f32, name=f"pmax4g{g}") for g in range(NG)]
        M4 = [pool.tile([128, GB], f32, name=f"M4g{g}") for g in range(NG)]
        mpr4 = [pool.tile([128, GB], f32, name=f"mpr4g{g}") for g in range(NG)]
        R4 = [pool.tile([128, GB], f32, name=f"R4g{g}") for g in range(NG)]
        F4 = [pool.tile([128, GB], f32, name=f"F4g{g}") for g in range(NG)]
        mrow4 = [pool.tile([128, GB], f32, name=f"mrow4g{g}") for g in range(NG)]
        mpjacc = [pool.tile([128, 1], f32, name=f"mpjaccg{g}") for g in range(NG)]
        ohs = [pool.tile([128, NCH * GB], f32, name=f"ohsg{g}") for g in range(NG)]
        oht = [pool.tile([128, NCH * GB], f32, name=f"ohtg{g}") for g in range(NG)]
        ohsacc = [pool.tile([128, 1], f32, name=f"ohsaccg{g}") for g in range(NG)]
        emitmask = pool.tile([128, SLOTS], f32)
        eprods = [pool.tile([128, SLOTS], f32, name=f"eprod{b}") for b in range(nb)]
        eprodsF = [pool.tile([128, SLOTS], f32, name=f"eprodF{b}") for b in range(nb)]
        t_col = pool.tile([128, 1], f32)

        PmP = pool.tile([128, 1], f32)
        ptmp = pool.tile([128, 1], i32)
        FmF = pool.tile([128, NCH], f32)
        fitmp = pool.tile([128, NCH], i32)
        iota2d = pool.tile([128, SLOTS], f32)
        i2tmp = pool.tile([128, SLOTS], i32)
        ones_r = pool.tile([1, 128], f32)

        # ---- constants / init ----
        nc.gpsimd.iota(ptmp[:], [[1, 1]], base=0, channel_multiplier=1)
        nc.vector.tensor_copy(PmP[:], ptmp[:])
        nc.vector.tensor_scalar(PmP[:], PmP[:], -1.0, 128.0, Alu.mult, Alu.add)
        nc.gpsimd.iota(fitmp[:], [[W, NCH]], base=0, channel_multiplier=1)
        nc.vector.tensor_copy(FmF[:], fitmp[:])
        nc.vector.tensor_scalar(FmF[:], FmF[:], -1.0, float(FREE), Alu.mult, Alu.add)
        nc.gpsimd.iota(i2tmp[:], [[1, SLOTS]], base=0, channel_multiplier=SLOTS)
        nc.vector.tensor_copy(iota2d[:], i2tmp[:])
        nc.any.memset(ones_r[:], 1.0)
        nc.any.memset(t_col[:], 0.0)
        for g in range(NG):
            nc.any.memset(R4[g][:], 128.0)       # encodes j=0: p*=0
            nc.any.memset(F4[g][:], float(FREE)) # encodes j=0: fi=0
            nc.any.memset(mrow4[g][:], 0.0)
            nc.any.memset(mrow4[g][0:1, :], 1.0)
            nc.any.memset(oht[g][:], 0.0)
            for bl in range(GB):
                nc.any.memset(oht[g][0:1, NCH * bl:NCH * bl + 1], 1.0)
        for b in range(nb):
            nc.any.memset(dist[b][:], 1e10)
            nc.any.memset(out2d[b][:], 0.0)
            nc.any.memset(out2dF[b][:], 0.0)

        # Contiguous load + on-chip de-interleave: a 12-byte-stride DMA of
        # each coordinate plane runs ~15x slower than a contiguous copy.
        craw = [pool.tile([128, 3 * FREE], f32, tag="craw", name=f"craw{b}",
                          bufs=2) for b in range(nb)]
        for b in range(nb):
            nc.sync.dma_start(craw[b][:, :], pred[b:b + 1, :, :])
            cr3 = craw[b].rearrange("p (f c) -> p c f", c=3)
            for c in range(3):
                nc.any.tensor_copy(C[b][:, c * FREE:(c + 1) * FREE], cr3[:, c, :])

        def m1(b, score_src):
            g, bl = b // GB, b % GB
            wps = psum.tile([128, NC4], f32, tag="wps", name=f"wps{b}")
            for k in range(3 * NCH):
                nc.tensor.matmul(
                    wps[0:W, k:k + 1], C[b][:, W * k:W * (k + 1)],
                    mrow4[g][:, bl:bl + 1], start=True, stop=True,
                )
            for h in range(NCH):
                nc.tensor.matmul(
                    wps[0:W, 3 * NCH + h:3 * NCH + h + 1],
                    score_src[b][:, W * h:W * (h + 1)],
                    mrow4[g][:, bl:bl + 1], start=True, stop=True,
                )
            nc.scalar.activation(wr[b][0:W, 0:3 * NCH], wps[0:W, 0:3 * NCH],
                                 Act.Copy)
            return wps

        def m2(b):
            g, bl = b // GB, b % GB
            for h in range(NCH):
                ohb = oht[g][0:W, NCH * bl + h:NCH * bl + h + 1]
                nc.tensor.matmul(
                    bcp[b][:, :], ohb.broadcast_to([W, 128]),
                    wr[b][0:W, h:3 * NCH:NCH],
                    start=(h == 0), stop=(h == NCH - 1),
                )

        # initial centroid: mrow/oht preset to point 0; score chunks unused ->
        # use dist as harmless stand-in for score_src
        for b in range(nb):
            m1(b, dist)
            m2(b)

        V = nc.vector
        G = nc.gpsimd

        def body(iv, u):
            # emit current farthest, encoded as (R, F); host decodes
            # j = (128-R)*FREE + (FREE-F)
            V.tensor_scalar(
                emitmask[:, :], iota2d[:, :], t_col[:, 0:1], None, Alu.is_equal
            )
            for b in range(nb):
                g, bl = b // GB, b % GB
                nc.scalar.activation(
                    eprods[b][:, :], emitmask[:, :], Act.Copy,
                    scale=R4[g][:, bl:bl + 1],
                )
                G.tensor_tensor(
                    out2d[b][:, :], out2d[b][:, :], eprods[b][:, :], Alu.add
                )
                nc.scalar.activation(
                    eprodsF[b][:, :], emitmask[:, :], Act.Copy,
                    scale=F4[g][:, bl:bl + 1],
                )
                G.tensor_tensor(
                    out2dF[b][:, :], out2dF[b][:, :], eprodsF[b][:, :], Alu.add
                )
            nc.scalar.activation(t_col[:, :], t_col[:, :], Act.Copy, bias=1.0)
            for g in range(NG):
                bs = list(range(g * GB, (g + 1) * GB))
                # distance + min (3 fused DVE passes per batch)
                for b in bs:
                    bl = b % GB
                    X = C[b][:, 0 * FREE:1 * FREE]
                    Y = C[b][:, 1 * FREE:2 * FREE]
                    Z = C[b][:, 2 * FREE:3 * FREE]
                    V._custom_dve(ops["SQDIFF2"], out=U[b][:], in0=X, in1=Y,
                                  s0=bcp[b][:, 0:1], s1=bcp[b][:, 1:2])
                    V._custom_dve(ops["SQADD"], out=Dd[b][:], in0=Z, in1=U[b][:],
                                  s0=bcp[b][:, 2:3])
                    V._custom_dve(ops["MINMAX"], out=dist[b][:], in0=dist[b][:],
                                  in1=Dd[b][:], accum_out=pmax4[g][:, bl:bl + 1])
                # cross-partition winner: max value, smallest partition on ties
                G.partition_all_reduce(M4[g][:, :], pmax4[g][:, :], channels=128,
                                       reduce_op=RO.max)
                V._custom_dve(ops["EQSCORE"], out=mpr4[g][:, :],
                              in0=pmax4[g][:, :], in1=PmP[:, :],
                              s0=M4[g][:, 0:1], accum_out=mpjacc[g][:, :])
                G.partition_all_reduce(R4[g][:, :], mpr4[g][:, :], channels=128,
                                       reduce_op=RO.max)
                V.tensor_tensor(mrow4[g][:, :], mpr4[g][:, :], R4[g][:, :],
                                Alu.is_equal)
                # winning row (coords + dist) via PE; within-row first max
                for b in bs:
                    bl = b % GB
                    wps = m1(b, dist)
                    V._custom_dve(ops["EQSCORE"],
                                  out=ohs[g][0:W, NCH * bl:NCH * bl + NCH],
                                  in0=wps[0:W, 3 * NCH:3 * NCH + NCH],
                                  in1=FmF[0:W, :], s0=M4[g][0:W, bl:bl + 1],
                                  accum_out=ohsacc[g][0:W, :])
                G.partition_all_reduce(F4[g][0:W, :], ohsacc[g][0:W, :],
                                       channels=W, reduce_op=RO.max)
                if W < 128:
                    G.partition_broadcast(F4[g][:, :], F4[g][0:1, :])
                for b in bs:
                    bl = b % GB
                    V.tensor_scalar(
                        oht[g][0:W, NCH * bl:NCH * bl + NCH],
                        ohs[g][0:W, NCH * bl:NCH * bl + NCH],
                        F4[g][0:W, bl:bl + 1], None, Alu.is_equal,
                    )
                    m2(b)

        assert NPTS % SLOTS == 0 and (NPTS <= MAXU or MAXU % SLOTS == 0)

        def unrollable_body(iv0, unroll):
            assert unroll % SLOTS == 0 or unroll <= SLOTS
            for i in range(unroll):
                body(iv0 + i, i)

        tc.For_i_unrolled_general(
            0, NPTS, 1, unrollable_body, max_unroll=MAXU,
            hint_engines=(mybir.EngineType.PE, mybir.EngineType.Activation,
                          mybir.EngineType.Pool, mybir.EngineType.DVE),
        )

        for b in range(nb):
            nc.vector.tensor_copy(outi[b][:, 0:SLOTS], out2d[b][:])
            nc.vector.tensor_copy(outi[b][:, SLOTS:2 * SLOTS], out2dF[b][:])
            nc.sync.dma_start(out_idx_d[b:b + 1, 0, :], outi[b][0:PMAX, 0:SLOTS])
            nc.sync.dma_start(out_idx_d[b:b + 1, 1, :], outi[b][0:PMAX, SLOTS:2 * SLOTS])


# ----------------------------------------------------------------------------
# Self-contained kernel entry point: full inputs in, full outputs out.
# ----------------------------------------------------------------------------

NB = 4          # batches per core
N_PTS = 32768   # points per cloud
NPTS_OUT = 1024
NCORES = 8

_NC_CACHE = {}


def _get_nc():
    if "nc" in _NC_CACHE:
        return _NC_CACHE["nc"]
    from concourse import bacc, tile as _tile

    nc = bacc.Bacc("TRN2", target_bir_lowering=False, debug=False)
    pred = nc.dram_tensor(
        "pred_cloud", [NB, N_PTS, 3], mybir.dt.float32, kind="ExternalInput"
    ).ap()
    out = nc.dram_tensor(
        "out", [NB, 2, NPTS_OUT], mybir.dt.int32, kind="ExternalOutput"
    ).ap()
    with _tile.TileContext(nc) as tc:
        build_fps(tc, out, pred, NB, N_PTS, NPTS_OUT)
    nc.compile()
    _NC_CACHE["nc"] = nc
    return nc


def kernel(pred_cloud):
    """pred_cloud [32, 32768, 3] f32 -> sampled points [32, 1024, 3] f32."""
    from concourse import bass_utils

    pred_cloud = np.ascontiguousarray(np.asarray(pred_cloud, dtype=np.float32))
    assert pred_cloud.shape == (NB * NCORES, N_PTS, 3)
    nc = _get_nc()
    in_maps = [
        {"pred_cloud": np.ascontiguousarray(pred_cloud[NB * i:NB * (i + 1)])}
        for i in range(NCORES)
    ]
    res = bass_utils.run_bass_kernel_spmd(nc, in_maps, core_ids=list(range(NCORES)))
    idx = np.concatenate(
        [decode_rf(res.results[i]["out"], N_PTS) for i in range(NCORES)], axis=0
    )  # [32, 1024] int64
    out = np.take_along_axis(pred_cloud, idx[:, :, None], axis=1)
    return np.ascontiguousarray(out.astype(np.float32))

